# revision 13
# baseline (speedup 1.0000x reference)
"""Trainium2 Bass kernel for dense flash-attention style GNN message passing.

Strategy (receiver-range sharding, 8 cores):
  - Edges sorted by receiver on host; core c owns receivers [1250c, 1250(c+1)).
  - Math rewrite: per-head projections commute with the softmax-weighted
    scatter, so each core only gathers bf16 rows [x(64) | er(8) | pad] of
    senders (256B rows via dma_gather) where er[n,h] = x[n]·acat[:,h] are
    pre-projected softmax logits built on device in a bulk phase 0 and
    written into the DRAM gather table; exp is applied per edge (fused with
    the per-edge length term for the radial heads).
  - Segment softmax without max-subtraction (logits are O(10), exact in f32);
    receiver-side logit terms are segment-constant and cancel.
  - Per 128-edge block: host-built one-hot S[e, r_local] (fp8, exact) as the
    stationary matmul operand; PE accumulates G[r, (h,f)] += S^T @
    (a_h[e] * x[s_e, f]) (bf16 moving, 1 cyc/row) and D[r, h] += S^T @ a
    over a whole 128-receiver tile in PSUM; epilogue normalizes by 1/D
    (fast approx reciprocal), applies the stacked head weight matrices, the
    degree-mask correction and w_out.
  - xe gather table uses a partition-major node remap n' = (n%128)*80 + n//128
    so the device fills its logit slots with strided 1024-descriptor DMAs.
  - The a*x expansion runs on DVE in 2x mode: per-head TTs with the f dim
    split (32, 2) and the a operand pre-doubled so every innermost AP dim is
    a packed 2-byte pair.
"""

import os
import sys

import numpy as np

N = 10000
E = 160000
F = 64
H = 4
NCORES = 8
RPC = N // NCORES          # receivers per core
TILES = 10                 # ceil(1250/128)
NPAD = 10240               # padded node count (80 blocks of 128)
NBLK = NPAD // 128         # 80


def _softplus(v):
    return np.logaddexp(0.0, np.asarray(v, np.float64)).astype(np.float32)


def _import_concourse():
    try:
        import concourse.bass  # noqa: F401
    except ImportError:
        for p in ("/opt/trn_rl_repo", "/root/.axon_site/_ro/trn_rl_repo"):
            if os.path.isdir(p) and p not in sys.path:
                sys.path.insert(0, p)
        import concourse.bass  # noqa: F401


def _bf16():
    import ml_dtypes
    return ml_dtypes.bfloat16


def _fp8():
    import ml_dtypes
    return ml_dtypes.float8_e4m3


def _split_multiwaits(nc, limit=1):
    """Walrus codegen can't encode more than ~2 sem waits on some
    instructions (e.g. the Tile tail InstDrain). Hoist excess waits onto
    InstNoOp instructions inserted immediately before the offender."""
    import concourse.mybir as mybir

    n_new = 0
    for bbname, bbobj in list(nc.bb_map.items()):
        bb = bbobj.bb if hasattr(bbobj, "bb") else bbobj
        try:
            insts = list(bb.instructions)
        except Exception:
            continue
        changed = False
        newlist = []
        for inst in insts:
            si = inst.sync_info
            if si is not None and si.on_wait and len(si.on_wait) > limit:
                waits = list(si.on_wait)
                rest, keep = waits[:-limit], waits[-limit:]
                for j in range(0, len(rest), limit):
                    nop = mybir.InstNoOp(
                        name=f"I-ws-{bbname}-{n_new}",
                        sync_info=mybir.SyncInfo(
                            on_wait=rest[j : j + limit], on_update=[]
                        ),
                        engine=inst.engine,
                        bass_nofuse=True,
                    )
                    n_new += 1
                    newlist.append(nop)
                    nc.register_instruction(nop)
                si.on_wait = keep
                changed = True
            newlist.append(inst)
        if changed:
            bb.instructions = newlist
    return n_new


def _build_program(NB):
    """Build the SPMD Bass program (uniform across cores). NB = blocks of 128
    edges per 128-receiver tile."""
    import concourse.bacc as bacc
    import concourse.mybir as mybir
    import concourse.tile as tile

    dt = mybir.dt
    AF = mybir.ActivationFunctionType
    OP = mybir.AluOpType

    nc = bacc.Bacc("TRN2")

    def din(name, shape, d=dt.float32):
        return nc.declare_dram_parameter(name, shape, d, isOutput=False)

    f32 = dt.float32
    bf16 = dt.bfloat16

    xe0_d = din("xe0", [128, NBLK * 128], bf16)   # preformatted [x|0|0] rows
    xT_d = din("xT", [F, NPAD], bf16)
    xtr_d = din("xtr", [F, TILES * 128], bf16)
    xr_d = din("xr", [128, TILES * F])
    snd_d = din("snd", [128, TILES * NB * 8], dt.int16)
    sall_d = din("sall", [128, TILES * NB * 128], dt.float8e4)
    len_d = din("lenv", [128, TILES * NB])
    deg_d = din("deg", [128, TILES])
    acat_d = din("acat", [F, 2 * H], bf16)
    c4t_d = din("c4t", [128, H])
    ident_d = din("ident", [128, 128], bf16)
    wst_d = din("wst", [128, 2 * H * F // 2], bf16)   # [128, 256]
    bbar_d = din("bbar", [F, F], bf16)
    wout_d = din("wout", [F, F], bf16)
    out_d = nc.declare_dram_parameter("out", [TILES * 128, F], f32, isOutput=True)
    # gather table, partition-major rows: row n' = p*NBLK + b at col b*128
    xe_d = nc.dram_tensor("xe", [128, NBLK * 128], bf16)

    with tile.TileContext(nc) as tc:
        with tc.tile_pool(name="const", bufs=1) as cp, \
             tc.tile_pool(name="gat", bufs=4) as gp, \
             tc.tile_pool(name="blk", bufs=2) as bp, \
             tc.tile_pool(name="ep", bufs=2) as ep, \
             tc.tile_pool(name="psg", bufs=2, space="PSUM") as psg, \
             tc.tile_pool(name="psd", bufs=2, space="PSUM") as psd, \
             tc.tile_pool(name="pst", bufs=2, space="PSUM") as pst, \
             tc.tile_pool(name="psf", bufs=2, space="PSUM") as psf:

            # ---- x rows into the gather table first (overlaps everything) --
            nc.sync.dma_start(xe_d[:, :], xe0_d[:, :])
            xe_bview = xe_d[:, :].rearrange("p (b e) -> p b e", e=128)

            # ---- constants into SBUF ----
            xT_sb = cp.tile([F, NPAD], bf16)
            nc.sync.dma_start(xT_sb[:], xT_d[:])
            acat_sb = cp.tile([F, 2 * H], bf16)
            nc.sync.dma_start(acat_sb[:], acat_d[:])
            snd_sb = cp.tile([128, TILES * NB * 8], dt.int16)
            nc.sync.dma_start(snd_sb[:], snd_d[:])
            sall_sb = cp.tile([128, TILES * NB * 128], dt.float8e4)
            nc.sync.dma_start(sall_sb[:], sall_d[:])
            len_sb = cp.tile([128, TILES * NB], f32)
            nc.sync.dma_start(len_sb[:], len_d[:])
            deg_sb = cp.tile([128, TILES], f32)
            nc.sync.dma_start(deg_sb[:], deg_d[:])
            c4t_sb = cp.tile([128, H], f32)
            nc.sync.dma_start(c4t_sb[:], c4t_d[:])
            ident_sb = cp.tile([128, 128], bf16)
            nc.sync.dma_start(ident_sb[:], ident_d[:])
            wst_sb = cp.tile([128, 256], bf16)
            nc.sync.dma_start(wst_sb[:], wst_d[:])
            bbar_sb = cp.tile([F, F], bf16)
            nc.sync.dma_start(bbar_sb[:], bbar_d[:])
            wout_sb = cp.tile([F, F], bf16)
            nc.sync.dma_start(wout_sb[:], wout_d[:])
            xtr_sb = cp.tile([F, TILES * 128], bf16)
            nc.sync.dma_start(xtr_sb[:], xtr_d[:])
            xr_sb = cp.tile([128, TILES, F], f32)
            nc.sync.dma_start(xr_sb[:], xr_d[:])

            # ---- phase 0: compute er logits, one bulk write into xe_d ----
            ers = cp.tile([128, NBLK, 2 * H], bf16)
            for g in range(NBLK // 8):
                er_ps = psf.tile([128, 8, 2 * H], f32, tag="F1")
                for j in range(8):
                    b = g * 8 + j
                    nc.tensor.matmul(
                        er_ps[:, j, :], xT_sb[:, b * 128 : (b + 1) * 128],
                        acat_sb[:], start=True, stop=True,
                    )
                nc.vector.tensor_scalar(
                    ers[:, g * 8 : (g + 1) * 8, :], er_ps[:], 0.0, None, OP.add
                )
            nc.sync.dma_start(xe_bview[:, :, F : F + 2 * H], ers[:])
            xe_gview = xe_d[:, :].rearrange("p (b e) -> (p b) e", e=128)

            # ---- phase 1: per receiver-tile ----
            for t in range(TILES):
                Xg = gp.tile([128, NB, 128], bf16, tag="Xg")
                # SWDGE ring caps a gather at 1024 idxs (8 blocks)
                for b0 in range(0, NB, 8):
                    b1 = min(b0 + 8, NB)
                    nc.gpsimd.dma_gather(
                        Xg[:, b0:b1, :], xe_gview,
                        snd_sb[:, t * NB * 8 + b0 * 8 : t * NB * 8 + b1 * 8],
                        (b1 - b0) * 128, (b1 - b0) * 128, 128,
                    )

                # a-factors (doubled pairs): ar8d[:, :, h, 0:2] = a_h
                lf = bp.tile([128, NB, H], f32, tag="lf")
                lsl = len_sb[:, t * NB : (t + 1) * NB]
                nc.vector.tensor_tensor(
                    lf[:],
                    lsl.unsqueeze(2).broadcast_to([128, NB, H]),
                    c4t_sb[:].unsqueeze(1).broadcast_to([128, NB, H]),
                    OP.mult,
                )
                arg = bp.tile([128, NB, H], f32, tag="arg")
                nc.vector.tensor_tensor(
                    arg[:], Xg[:, :, F : F + H], lf[:], OP.add
                )
                ar8d = bp.tile([128, NB, 2 * H, 2], bf16, tag="ar8d")
                for j in range(2):
                    nc.scalar.activation(
                        ar8d[:, :, 0:H, j], arg[:], AF.Exp)
                    nc.scalar.activation(
                        ar8d[:, :, H : 2 * H, j],
                        Xg[:, :, F + H : F + 2 * H], AF.Exp)

                # ex[e, nb, h, f] = a_h[e,nb] * x[e, nb, f]  (DVE 2x: inner
                # dims are packed pairs)
                ex = bp.tile([128, NB, 2 * H, F], bf16, tag="ex")
                xg_in = Xg[:, :, 0:F].rearrange("p nb (r two) -> p nb r two", two=2)
                for h in range(2 * H):
                    nc.vector.tensor_tensor(
                        ex[:, :, h, :].rearrange(
                            "p nb (r two) -> p nb r two", two=2),
                        ar8d[:, :, h, :].unsqueeze(2)
                            .broadcast_to([128, NB, F // 2, 2]),
                        xg_in,
                        OP.mult,
                    )

                G_ps = psg.tile([128, 512], f32, tag="G")
                D_ps = psd.tile([128, 2 * H], f32, tag="D")
                for b in range(NB):
                    S = sall_sb[:, (t * NB + b) * 128 : (t * NB + b + 1) * 128]
                    nc.tensor.matmul(
                        G_ps[:], S, ex[:, b, :, :],
                        start=(b == 0), stop=(b == NB - 1),
                    )
                    nc.tensor.matmul(
                        D_ps[:], S, ar8d[:, b, :, 0],
                        start=(b == 0), stop=(b == NB - 1),
                    )

                # ---- tile epilogue ----
                dsum = ep.tile([128, 2 * H], f32, tag="dsum")
                nc.vector.tensor_scalar(dsum[:], D_ps[:], 1e-30, None, OP.add)
                invd = ep.tile([128, 2 * H], f32, tag="invd")
                nc.vector.reciprocal_approx_fast(invd[:], dsum[:])
                gn = ep.tile([128, 2 * H, F], bf16, tag="gn")
                for h in range(2 * H):
                    sl = slice(h * F, (h + 1) * F)
                    if h % 2 == 0:
                        nc.vector.tensor_scalar(
                            gn[:, h, :], G_ps[:, sl], invd[:, h : h + 1],
                            None, OP.mult,
                        )
                    else:
                        nc.scalar.activation(
                            gn[:, h, :], G_ps[:, sl], AF.Copy,
                            scale=invd[:, h : h + 1],
                        )
                gnT = ep.tile([128, 4, 128], bf16, tag="gnT")
                for k in range(4):
                    tp_ps = pst.tile([128, 128], bf16, tag="tp")
                    nc.tensor.transpose(
                        tp_ps[:], gn[:, 2 * k : 2 * k + 2, :], ident_sb[:]
                    )
                    if k % 2 == 0:
                        nc.vector.tensor_scalar(
                            gnT[:, k, :], tp_ps[:], 0.0, None, OP.add
                        )
                    else:
                        nc.scalar.activation(gnT[:, k, :], tp_ps[:], AF.Copy)
                F1_ps = psf.tile([128, F], f32, tag="F1")
                for k in range(4):
                    nc.tensor.matmul(
                        F1_ps[:], gnT[:, k, :], wst_sb[:, k * F : (k + 1) * F],
                        start=(k == 0), stop=(k == 3),
                    )
                F2_ps = psf.tile([128, F], f32, tag="F1")
                nc.tensor.matmul(
                    F2_ps[:], xtr_sb[:, t * 128 : (t + 1) * 128], bbar_sb[:],
                    start=True, stop=True,
                )
                m1 = ep.tile([128, F], bf16, tag="m1")
                nc.vector.tensor_scalar(
                    m1[:], F2_ps[:], deg_sb[:, t : t + 1], None, OP.mult
                )
                msg = ep.tile([128, F], bf16, tag="msg")
                nc.vector.tensor_tensor(msg[:], F1_ps[:], m1[:], OP.subtract)
                tp2_ps = pst.tile([F, 128], bf16, tag="tp")
                nc.tensor.transpose(tp2_ps[:], msg[:], ident_sb[:])
                msgT = ep.tile([F, 128], bf16, tag="msgT")
                nc.scalar.activation(msgT[:], tp2_ps[:], AF.Copy)
                F3_ps = psf.tile([128, F], f32, tag="F1")
                nc.tensor.matmul(F3_ps[:], msgT[:], wout_sb[:], start=True, stop=True)
                ot = ep.tile([128, F], f32, tag="ot")
                nc.vector.tensor_tensor(ot[:], F3_ps[:], xr_sb[:, t, :], OP.add)
                nc.sync.dma_start(out_d[t * 128 : (t + 1) * 128, :], ot[:])

    nc.compile()
    _split_multiwaits(nc, limit=1)
    return nc


def _host_prep(x, edge_index, edge_len, w_proj_W, radial_W, tangential_W,
               radial_score, tangential_score, radial_distance_log_scale,
               radial_temp_bias, radial_temp_weight, w_out_W):
    bf = _bf16()
    f8 = _fp8()
    x = np.asarray(x, np.float32)
    sender = np.asarray(edge_index[0], np.int64)
    receiver = np.asarray(edge_index[1], np.int64)
    edge_len = np.asarray(edge_len, np.float32)

    order = np.argsort(receiver, kind="stable")
    s_s = sender[order].astype(np.int32)
    r_s = receiver[order].astype(np.int32)
    l_s = edge_len[order]
    # partition-major node remap for the xe gather table
    s_rm = (s_s % 128) * NBLK + s_s // 128

    # per (core, tile) edge spans
    spans = []
    NBmax = 1
    for c in range(NCORES):
        for t in range(TILES):
            base = c * RPC + t * 128
            top = min(base + 128, (c + 1) * RPC)
            lo = np.searchsorted(r_s, base)
            hi = np.searchsorted(r_s, top)
            spans.append((c, t, base, lo, hi))
            NBmax = max(NBmax, -(-(hi - lo) // 128))
    NB = int(NBmax)

    deg = np.bincount(receiver, minlength=N)

    snd_all = np.zeros((NCORES, 128, TILES * NB * 8), np.int16)
    sall_all = np.zeros((NCORES, 128, TILES * NB * 128), f8)
    len_all = np.zeros((NCORES, 128, TILES * NB), np.float32)
    deg_all = np.zeros((NCORES, 128, TILES), np.float32)

    one8 = f8(1.0)
    for (c, t, base, lo, hi) in spans:
        n_e = hi - lo
        snd = np.zeros(NB * 128, np.int32)
        snd[:n_e] = s_rm[lo:hi]
        lv = np.zeros(NB * 128, np.float32)
        lv[:n_e] = l_s[lo:hi]
        snd_all[c, :, t * NB * 8 : (t + 1) * NB * 8] = np.tile(
            snd.astype(np.int16).reshape(-1, 16).T, (8, 1)
        )
        len_all[c, :, t * NB : (t + 1) * NB] = lv.reshape(NB, 128).T
        # one-hot S: edge slot e (block b=e//128, partition p=e%128) ->
        # column (t*NB+b)*128 + (r - base)
        e_sl = np.arange(n_e)
        p_idx = e_sl % 128
        col = (t * NB + e_sl // 128) * 128 + (r_s[lo:hi] - base)
        sall_all[c, p_idx, col] = one8
    for c in range(NCORES):
        dv = np.zeros(TILES * 128, np.float32)
        dv[:RPC] = (deg[c * RPC : (c + 1) * RPC] > 0).astype(np.float32)
        deg_all[c] = dv.reshape(TILES, 128).T

    # weight folding (host: O(F^2 H) only)
    scale = float(_softplus(radial_distance_log_scale))
    temp = _softplus(np.asarray(radial_temp_bias, np.float32))  # [H]
    it = (1.0 / (temp + 1e-4)).astype(np.float32)

    acat = np.zeros((F, 2 * H), np.float32)
    for h in range(H):
        acat[:, h] = (w_proj_W[h] @ radial_score[h]) * it[h]
        acat[:, H + h] = w_proj_W[h] @ tangential_score[h]
    c4row = (-scale * it).astype(np.float32)
    c4t = np.tile(c4row, (128, 1))

    wstack = np.concatenate(
        [np.asarray(radial_W, np.float32).reshape(H * F, F),
         np.asarray(tangential_W, np.float32).reshape(H * F, F)], axis=0
    ) * (1.0 / H)
    wst = wstack.reshape(4, 128, F).transpose(1, 0, 2).reshape(128, 4 * F)
    bbar = (np.asarray(radial_W, np.float32).sum(0)
            + np.asarray(tangential_W, np.float32).sum(0)) * (1.0 / H)

    xpad = np.zeros((NPAD, F), np.float32)
    xpad[:N] = x
    # xe0: [p, b, 0:64] = x[b*128+p], rest zero
    xe0 = np.zeros((128, NBLK, 128), bf)
    xe0[:, :, 0:F] = xpad.reshape(NBLK, 128, F).transpose(1, 0, 2).astype(bf)
    xT = np.ascontiguousarray(xpad.T).astype(bf)
    ident = np.eye(128, dtype=np.float32).astype(bf)

    in_maps = []
    for c in range(NCORES):
        xw = xpad[c * RPC : c * RPC + TILES * 128]
        in_maps.append({
            "xe0": xe0.reshape(128, NBLK * 128),
            "xT": xT,
            "xtr": np.ascontiguousarray(xw.T).astype(bf),
            "xr": np.ascontiguousarray(
                xw.reshape(TILES, 128, F).transpose(1, 0, 2)
            ).reshape(128, TILES * F),
            "snd": snd_all[c],
            "sall": sall_all[c],
            "lenv": len_all[c],
            "deg": deg_all[c],
            "acat": acat.astype(bf),
            "c4t": c4t,
            "ident": ident,
            "wst": wst.astype(bf),
            "bbar": bbar.astype(bf),
            "wout": np.asarray(w_out_W, np.float32).astype(bf),
        })
    return NB, in_maps


def _numpy_fallback(x, edge_index, edge_vec, edge_len, w_proj_W, radial_W,
                    tangential_W, radial_score, tangential_score,
                    radial_distance_log_scale, radial_temp_bias,
                    radial_temp_weight, w_out_W):
    # general-path fallback (not exercised by the shipped inputs)
    x = np.asarray(x, np.float64)
    snd, rcv = np.asarray(edge_index[0]), np.asarray(edge_index[1])
    energy = np.einsum("nf,hfg->hng", x, np.asarray(w_proj_W, np.float64))
    radial = np.einsum("nf,hfg->hng", x, np.asarray(radial_W, np.float64))
    tang = np.einsum("nf,hfg->hng", x, np.asarray(tangential_W, np.float64))
    e_d = energy[:, snd] - energy[:, rcv]
    r_d = (radial[:, snd] - radial[:, rcv]).transpose(1, 0, 2)
    t_d = (tang[:, snd] - tang[:, rcv]).transpose(1, 0, 2)
    sp = lambda v: np.logaddexp(0.0, v)
    scale = sp(np.asarray(radial_distance_log_scale, np.float64))
    r_log = (np.einsum("hef,hf->eh", e_d, np.asarray(radial_score, np.float64))
             - scale * np.asarray(edge_len, np.float64)[:, None])
    temp = sp(np.asarray(radial_temp_bias, np.float64)[None, :]
              + np.asarray(radial_temp_weight, np.float64)[None, :]
              * np.asarray(edge_len, np.float64)[:, None])
    r_log = r_log / (temp + 1e-4)
    t_log = np.einsum("hef,hf->eh", e_d, np.asarray(tangential_score, np.float64))

    def seg_softmax(logits):
        m = np.full((N, H), -np.inf)
        np.maximum.at(m, rcv, logits)
        a = np.exp(logits - m[rcv])
        d = np.zeros((N, H))
        np.add.at(d, rcv, a)
        return a / d[rcv]

    r_a, t_a = seg_softmax(r_log), seg_softmax(t_log)
    r_msg = np.zeros((N, H, F)); np.add.at(r_msg, rcv, r_a[:, :, None] * r_d)
    t_msg = np.zeros((N, H, F)); np.add.at(t_msg, rcv, t_a[:, :, None] * t_d)
    out = np.nan_to_num((r_msg + t_msg).mean(axis=1))
    return (x + out @ np.asarray(w_out_W, np.float64)).astype(np.float32)


_PROG_CACHE = {}


def kernel(**inputs) -> np.ndarray:
    if not np.allclose(np.asarray(inputs["radial_temp_weight"]), 0.0):
        return _numpy_fallback(**inputs)

    _import_concourse()
    from concourse.bass_utils import run_bass_kernel_spmd

    NB, in_maps = _host_prep(
        inputs["x"], inputs["edge_index"], inputs["edge_len"],
        inputs["w_proj_W"], inputs["radial_W"], inputs["tangential_W"],
        inputs["radial_score"], inputs["tangential_score"],
        inputs["radial_distance_log_scale"], inputs["radial_temp_bias"],
        inputs["radial_temp_weight"], inputs["w_out_W"],
    )
    if NB not in _PROG_CACHE:
        _PROG_CACHE[NB] = _build_program(NB)
    nc = _PROG_CACHE[NB]

    core_ids = list(range(NCORES))
    res = run_bass_kernel_spmd(nc, in_maps, core_ids)
    outs = [np.asarray(res.results[c]["out"])[:RPC] for c in core_ids]
    return np.concatenate(outs, axis=0).astype(np.float32)


if __name__ == "__main__":
    rng = np.random.default_rng(0)
    print("kernel module ok")


# revision 16
# speedup vs baseline: 1.2002x; 1.2002x over previous
"""Trainium2 Bass kernel for dense flash-attention style GNN message passing.

Strategy (receiver-range sharding, 8 cores):
  - Edges sorted by receiver on host; core c owns receivers [1250c, 1250(c+1)).
  - Math rewrite: per-head projections commute with the softmax-weighted
    scatter, so each core only gathers bf16 rows [x(64) | er(8) | pad] of
    senders (256B rows via dma_gather) where er[n,h] = x[n]·acat[:,h] are
    pre-projected softmax logits built on device in a bulk phase 0 and
    written into the DRAM gather table; exp is applied per edge (fused with
    the per-edge length term for the radial heads).
  - Segment softmax without max-subtraction (logits are O(10), exact in f32);
    receiver-side logit terms are segment-constant and cancel.
  - Per 128-edge block: host-built one-hot S[e, r_local] (fp8, exact) as the
    stationary matmul operand; PE accumulates G[r, (h,f)] += S^T @
    (a_h[e] * x[s_e, f]) (bf16 moving, 1 cyc/row) and D[r, h] += S^T @ a
    over a whole 128-receiver tile in PSUM; epilogue normalizes by 1/D
    (fast approx reciprocal), applies the stacked head weight matrices, the
    degree-mask correction and w_out.
  - xe gather table uses a partition-major node remap n' = (n%128)*80 + n//128
    so the device fills its logit slots with strided 1024-descriptor DMAs.
  - The a*x expansion runs on DVE in 2x mode: per-head TTs with the f dim
    split (32, 2) and the a operand pre-doubled so every innermost AP dim is
    a packed 2-byte pair.
"""

import os
import sys

import numpy as np

N = 10000
E = 160000
F = 64
H = 4
NCORES = 8
RPC = N // NCORES          # receivers per core
TILES = 10                 # ceil(1250/128)
NPAD = 10240               # padded node count (80 blocks of 128)
NBLK = NPAD // 128         # 80


def _softplus(v):
    return np.logaddexp(0.0, np.asarray(v, np.float64)).astype(np.float32)


def _import_concourse():
    try:
        import concourse.bass  # noqa: F401
    except ImportError:
        for p in ("/opt/trn_rl_repo", "/root/.axon_site/_ro/trn_rl_repo"):
            if os.path.isdir(p) and p not in sys.path:
                sys.path.insert(0, p)
        import concourse.bass  # noqa: F401


def _bf16():
    import ml_dtypes
    return ml_dtypes.bfloat16


def _fp8():
    import ml_dtypes
    return ml_dtypes.float8_e4m3


def _split_multiwaits(nc, limit=1):
    """Walrus codegen can't encode more than ~2 sem waits on some
    instructions (e.g. the Tile tail InstDrain). Hoist excess waits onto
    InstNoOp instructions inserted immediately before the offender."""
    import concourse.mybir as mybir

    n_new = 0
    for bbname, bbobj in list(nc.bb_map.items()):
        bb = bbobj.bb if hasattr(bbobj, "bb") else bbobj
        try:
            insts = list(bb.instructions)
        except Exception:
            continue
        changed = False
        newlist = []
        for inst in insts:
            si = inst.sync_info
            if si is not None and si.on_wait and len(si.on_wait) > limit:
                waits = list(si.on_wait)
                rest, keep = waits[:-limit], waits[-limit:]
                for j in range(0, len(rest), limit):
                    nop = mybir.InstNoOp(
                        name=f"I-ws-{bbname}-{n_new}",
                        sync_info=mybir.SyncInfo(
                            on_wait=rest[j : j + limit], on_update=[]
                        ),
                        engine=inst.engine,
                        bass_nofuse=True,
                    )
                    n_new += 1
                    newlist.append(nop)
                    nc.register_instruction(nop)
                si.on_wait = keep
                changed = True
            newlist.append(inst)
        if changed:
            bb.instructions = newlist
    return n_new


def _build_program(NB):
    """Build the SPMD Bass program (uniform across cores). NB = blocks of 128
    edges per 128-receiver tile."""
    import concourse.bacc as bacc
    import concourse.mybir as mybir
    import concourse.tile as tile

    dt = mybir.dt
    AF = mybir.ActivationFunctionType
    OP = mybir.AluOpType

    nc = bacc.Bacc("TRN2")

    def din(name, shape, d=dt.float32):
        return nc.declare_dram_parameter(name, shape, d, isOutput=False)

    f32 = dt.float32
    bf16 = dt.bfloat16

    xe0_d = din("xe0", [128, NBLK * 128], bf16)   # preformatted [x|0|0] rows
    xT_d = din("xT", [F, NPAD], bf16)
    xtr_d = din("xtr", [F, TILES * 128], bf16)
    xr_d = din("xr", [128, TILES * F])
    snd_d = din("snd", [128, TILES * NB * 8], dt.int16)
    sall_d = din("sall", [128, TILES * NB * 128], dt.float8e4)
    len_d = din("lenv", [128, TILES * NB])
    deg_d = din("deg", [128, TILES])
    acat_d = din("acat", [F, 2 * H], bf16)
    c4t_d = din("c4t", [128, H])
    ident_d = din("ident", [128, 128], bf16)
    wst_d = din("wst", [128, 2 * H * F // 2], bf16)   # [128, 256]
    bbar_d = din("bbar", [F, F], bf16)
    wout_d = din("wout", [F, F], bf16)
    out_d = nc.declare_dram_parameter("out", [TILES * 128, F], f32, isOutput=True)
    # gather table, partition-major rows: row n' = p*NBLK + b at col b*128
    xe_d = nc.dram_tensor("xe", [128, NBLK * 128], bf16)

    with tile.TileContext(nc) as tc:
        with tc.tile_pool(name="const", bufs=1) as cp, \
             tc.tile_pool(name="gat", bufs=3) as gp, \
             tc.tile_pool(name="blk", bufs=2) as bp, \
             tc.tile_pool(name="ep", bufs=2) as ep, \
             tc.tile_pool(name="psg", bufs=2, space="PSUM") as psg, \
             tc.tile_pool(name="psd", bufs=2, space="PSUM") as psd, \
             tc.tile_pool(name="pst", bufs=2, space="PSUM") as pst, \
             tc.tile_pool(name="psf", bufs=2, space="PSUM") as psf:

            # ---- constants into SBUF ----
            xT_sb = cp.tile([F, NPAD], bf16)
            nc.sync.dma_start(xT_sb[:], xT_d[:])
            xe_sb = cp.tile([128, NBLK, 128], bf16)
            nc.sync.dma_start(xe_sb[:], xe0_d[:])
            xe_bview = xe_d[:, :].rearrange("p (b e) -> p b e", e=128)
            acat_sb = cp.tile([F, 2 * H], bf16)
            nc.sync.dma_start(acat_sb[:], acat_d[:])
            snd_sb = cp.tile([128, TILES * NB * 8], dt.int16)
            nc.sync.dma_start(snd_sb[:], snd_d[:])
            sall_sb = cp.tile([128, TILES * NB * 128], dt.float8e4)
            nc.sync.dma_start(sall_sb[:], sall_d[:])
            len_sb = cp.tile([128, TILES * NB], f32)
            nc.sync.dma_start(len_sb[:], len_d[:])
            deg_sb = cp.tile([128, TILES], f32)
            nc.sync.dma_start(deg_sb[:], deg_d[:])
            c4t_sb = cp.tile([128, H], f32)
            nc.sync.dma_start(c4t_sb[:], c4t_d[:])
            ident_sb = cp.tile([128, 128], bf16)
            nc.sync.dma_start(ident_sb[:], ident_d[:])
            wst_sb = cp.tile([128, 256], bf16)
            nc.sync.dma_start(wst_sb[:], wst_d[:])
            bbar_sb = cp.tile([F, F], bf16)
            nc.sync.dma_start(bbar_sb[:], bbar_d[:])
            wout_sb = cp.tile([F, F], bf16)
            nc.sync.dma_start(wout_sb[:], wout_d[:])
            xtr_sb = cp.tile([F, TILES * 128], bf16)
            nc.sync.dma_start(xtr_sb[:], xtr_d[:])
            xr_sb = cp.tile([128, TILES, F], f32)
            nc.sync.dma_start(xr_sb[:], xr_d[:])

            # ---- phase 0: er logits into xe_sb rows, write groups to xe_d --
            for g in range(NBLK // 8):
                er_ps = psf.tile([128, 8, 2 * H], f32, tag="F1")
                for j in range(8):
                    b = g * 8 + j
                    nc.tensor.matmul(
                        er_ps[:, j, :], xT_sb[:, b * 128 : (b + 1) * 128],
                        acat_sb[:], start=True, stop=True,
                    )
                nc.vector.tensor_scalar(
                    xe_sb[:, g * 8 : (g + 1) * 8, F : F + 2 * H],
                    er_ps[:], 0.0, None, OP.add,
                )
                nc.sync.dma_start(
                    xe_bview[:, g * 8 : (g + 1) * 8, :],
                    xe_sb[:, g * 8 : (g + 1) * 8, :],
                )
            xe_gview = xe_d[:, :].rearrange("p (b e) -> (p b) e", e=128)

            # ---- phase 1: per receiver-tile ----
            for t in range(TILES):
                Xg = gp.tile([128, NB, 128], bf16, tag="Xg")
                # SWDGE ring caps a gather at 1024 idxs (8 blocks)
                for b0 in range(0, NB, 8):
                    b1 = min(b0 + 8, NB)
                    nc.gpsimd.dma_gather(
                        Xg[:, b0:b1, :], xe_gview,
                        snd_sb[:, t * NB * 8 + b0 * 8 : t * NB * 8 + b1 * 8],
                        (b1 - b0) * 128, (b1 - b0) * 128, 128,
                    )

                # a-factors (doubled pairs): ar8d[:, :, h, 0:2] = a_h
                lf = bp.tile([128, NB, H], f32, tag="lf")
                lsl = len_sb[:, t * NB : (t + 1) * NB]
                nc.vector.tensor_tensor(
                    lf[:],
                    lsl.unsqueeze(2).broadcast_to([128, NB, H]),
                    c4t_sb[:].unsqueeze(1).broadcast_to([128, NB, H]),
                    OP.mult,
                )
                arg = bp.tile([128, NB, H], f32, tag="arg")
                nc.vector.tensor_tensor(
                    arg[:], Xg[:, :, F : F + H], lf[:], OP.add
                )
                ar8d = bp.tile([128, NB, 2 * H, 2], bf16, tag="ar8d")
                for j in range(2):
                    nc.scalar.activation(
                        ar8d[:, :, 0:H, j], arg[:], AF.Exp)
                    nc.scalar.activation(
                        ar8d[:, :, H : 2 * H, j],
                        Xg[:, :, F + H : F + 2 * H], AF.Exp)

                # ex[e, nb, h, f] = a_h[e,nb] * x[e, nb, f]  (DVE 2x: inner
                # dims are packed pairs)
                ex = bp.tile([128, NB, 2 * H, F], bf16, tag="ex")
                xg_in = Xg[:, :, 0:F].rearrange("p nb (r two) -> p nb r two", two=2)
                for h in range(2 * H):
                    nc.vector.tensor_tensor(
                        ex[:, :, h, :].rearrange(
                            "p nb (r two) -> p nb r two", two=2),
                        ar8d[:, :, h, :].unsqueeze(2)
                            .broadcast_to([128, NB, F // 2, 2]),
                        xg_in,
                        OP.mult,
                    )

                G_ps = psg.tile([128, 512], f32, tag="G")
                D_ps = psd.tile([128, 2 * H], f32, tag="D")
                for b in range(NB):
                    S = sall_sb[:, (t * NB + b) * 128 : (t * NB + b + 1) * 128]
                    nc.tensor.matmul(
                        G_ps[:], S, ex[:, b, :, :],
                        start=(b == 0), stop=(b == NB - 1),
                    )
                    nc.tensor.matmul(
                        D_ps[:], S, ar8d[:, b, :, 0],
                        start=(b == 0), stop=(b == NB - 1),
                    )

                # ---- tile epilogue ----
                dsum = ep.tile([128, 2 * H], f32, tag="dsum")
                nc.vector.tensor_scalar(dsum[:], D_ps[:], 1e-30, None, OP.add)
                invd = ep.tile([128, 2 * H], f32, tag="invd")
                nc.vector.reciprocal_approx_fast(invd[:], dsum[:])
                gn = ep.tile([128, 2 * H, F], bf16, tag="gn")
                for h in range(2 * H):
                    sl = slice(h * F, (h + 1) * F)
                    if h % 2 == 0:
                        nc.vector.tensor_scalar(
                            gn[:, h, :], G_ps[:, sl], invd[:, h : h + 1],
                            None, OP.mult,
                        )
                    else:
                        nc.scalar.activation(
                            gn[:, h, :], G_ps[:, sl], AF.Copy,
                            scale=invd[:, h : h + 1],
                        )
                gnT = ep.tile([128, 4, 128], bf16, tag="gnT")
                for k in range(4):
                    tp_ps = pst.tile([128, 128], bf16, tag="tp")
                    nc.tensor.transpose(
                        tp_ps[:], gn[:, 2 * k : 2 * k + 2, :], ident_sb[:]
                    )
                    if k % 2 == 0:
                        nc.vector.tensor_scalar(
                            gnT[:, k, :], tp_ps[:], 0.0, None, OP.add
                        )
                    else:
                        nc.scalar.activation(gnT[:, k, :], tp_ps[:], AF.Copy)
                F1_ps = psf.tile([128, F], f32, tag="F1")
                for k in range(4):
                    nc.tensor.matmul(
                        F1_ps[:], gnT[:, k, :], wst_sb[:, k * F : (k + 1) * F],
                        start=(k == 0), stop=(k == 3),
                    )
                F2_ps = psf.tile([128, F], f32, tag="F1")
                nc.tensor.matmul(
                    F2_ps[:], xtr_sb[:, t * 128 : (t + 1) * 128], bbar_sb[:],
                    start=True, stop=True,
                )
                m1 = ep.tile([128, F], bf16, tag="m1")
                nc.vector.tensor_scalar(
                    m1[:], F2_ps[:], deg_sb[:, t : t + 1], None, OP.mult
                )
                msg = ep.tile([128, F], bf16, tag="msg")
                nc.vector.tensor_tensor(msg[:], F1_ps[:], m1[:], OP.subtract)
                tp2_ps = pst.tile([F, 128], bf16, tag="tp")
                nc.tensor.transpose(tp2_ps[:], msg[:], ident_sb[:])
                msgT = ep.tile([F, 128], bf16, tag="msgT")
                nc.scalar.activation(msgT[:], tp2_ps[:], AF.Copy)
                F3_ps = psf.tile([128, F], f32, tag="F1")
                nc.tensor.matmul(F3_ps[:], msgT[:], wout_sb[:], start=True, stop=True)
                ot = ep.tile([128, F], f32, tag="ot")
                nc.vector.tensor_tensor(ot[:], F3_ps[:], xr_sb[:, t, :], OP.add)
                nc.sync.dma_start(out_d[t * 128 : (t + 1) * 128, :], ot[:])

    nc.compile()
    _split_multiwaits(nc, limit=1)
    return nc


def _host_prep(x, edge_index, edge_len, w_proj_W, radial_W, tangential_W,
               radial_score, tangential_score, radial_distance_log_scale,
               radial_temp_bias, radial_temp_weight, w_out_W):
    bf = _bf16()
    f8 = _fp8()
    x = np.asarray(x, np.float32)
    sender = np.asarray(edge_index[0], np.int64)
    receiver = np.asarray(edge_index[1], np.int64)
    edge_len = np.asarray(edge_len, np.float32)

    order = np.argsort(receiver, kind="stable")
    s_s = sender[order].astype(np.int32)
    r_s = receiver[order].astype(np.int32)
    l_s = edge_len[order]
    # partition-major node remap for the xe gather table
    s_rm = (s_s % 128) * NBLK + s_s // 128

    # per (core, tile) edge spans
    spans = []
    NBmax = 1
    for c in range(NCORES):
        for t in range(TILES):
            base = c * RPC + t * 128
            top = min(base + 128, (c + 1) * RPC)
            lo = np.searchsorted(r_s, base)
            hi = np.searchsorted(r_s, top)
            spans.append((c, t, base, lo, hi))
            NBmax = max(NBmax, -(-(hi - lo) // 128))
    NB = int(NBmax)

    deg = np.bincount(receiver, minlength=N)

    snd_all = np.zeros((NCORES, 128, TILES * NB * 8), np.int16)
    sall_all = np.zeros((NCORES, 128, TILES * NB * 128), f8)
    len_all = np.zeros((NCORES, 128, TILES * NB), np.float32)
    deg_all = np.zeros((NCORES, 128, TILES), np.float32)

    one8 = f8(1.0)
    for (c, t, base, lo, hi) in spans:
        n_e = hi - lo
        snd = np.zeros(NB * 128, np.int32)
        snd[:n_e] = s_rm[lo:hi]
        lv = np.zeros(NB * 128, np.float32)
        lv[:n_e] = l_s[lo:hi]
        snd_all[c, :, t * NB * 8 : (t + 1) * NB * 8] = np.tile(
            snd.astype(np.int16).reshape(-1, 16).T, (8, 1)
        )
        len_all[c, :, t * NB : (t + 1) * NB] = lv.reshape(NB, 128).T
        # one-hot S: edge slot e (block b=e//128, partition p=e%128) ->
        # column (t*NB+b)*128 + (r - base)
        e_sl = np.arange(n_e)
        p_idx = e_sl % 128
        col = (t * NB + e_sl // 128) * 128 + (r_s[lo:hi] - base)
        sall_all[c, p_idx, col] = one8
    for c in range(NCORES):
        dv = np.zeros(TILES * 128, np.float32)
        dv[:RPC] = (deg[c * RPC : (c + 1) * RPC] > 0).astype(np.float32)
        deg_all[c] = dv.reshape(TILES, 128).T

    # weight folding (host: O(F^2 H) only)
    scale = float(_softplus(radial_distance_log_scale))
    temp = _softplus(np.asarray(radial_temp_bias, np.float32))  # [H]
    it = (1.0 / (temp + 1e-4)).astype(np.float32)

    acat = np.zeros((F, 2 * H), np.float32)
    for h in range(H):
        acat[:, h] = (w_proj_W[h] @ radial_score[h]) * it[h]
        acat[:, H + h] = w_proj_W[h] @ tangential_score[h]
    c4row = (-scale * it).astype(np.float32)
    c4t = np.tile(c4row, (128, 1))

    wstack = np.concatenate(
        [np.asarray(radial_W, np.float32).reshape(H * F, F),
         np.asarray(tangential_W, np.float32).reshape(H * F, F)], axis=0
    ) * (1.0 / H)
    wst = wstack.reshape(4, 128, F).transpose(1, 0, 2).reshape(128, 4 * F)
    bbar = (np.asarray(radial_W, np.float32).sum(0)
            + np.asarray(tangential_W, np.float32).sum(0)) * (1.0 / H)

    xpad = np.zeros((NPAD, F), np.float32)
    xpad[:N] = x
    # xe0: [p, b, 0:64] = x[b*128+p], rest zero
    xe0 = np.zeros((128, NBLK, 128), bf)
    xe0[:, :, 0:F] = xpad.reshape(NBLK, 128, F).transpose(1, 0, 2).astype(bf)
    xT = np.ascontiguousarray(xpad.T).astype(bf)
    ident = np.eye(128, dtype=np.float32).astype(bf)

    in_maps = []
    for c in range(NCORES):
        xw = xpad[c * RPC : c * RPC + TILES * 128]
        in_maps.append({
            "xe0": xe0.reshape(128, NBLK * 128),
            "xT": xT,
            "xtr": np.ascontiguousarray(xw.T).astype(bf),
            "xr": np.ascontiguousarray(
                xw.reshape(TILES, 128, F).transpose(1, 0, 2)
            ).reshape(128, TILES * F),
            "snd": snd_all[c],
            "sall": sall_all[c],
            "lenv": len_all[c],
            "deg": deg_all[c],
            "acat": acat.astype(bf),
            "c4t": c4t,
            "ident": ident,
            "wst": wst.astype(bf),
            "bbar": bbar.astype(bf),
            "wout": np.asarray(w_out_W, np.float32).astype(bf),
        })
    return NB, in_maps


def _numpy_fallback(x, edge_index, edge_vec, edge_len, w_proj_W, radial_W,
                    tangential_W, radial_score, tangential_score,
                    radial_distance_log_scale, radial_temp_bias,
                    radial_temp_weight, w_out_W):
    # general-path fallback (not exercised by the shipped inputs)
    x = np.asarray(x, np.float64)
    snd, rcv = np.asarray(edge_index[0]), np.asarray(edge_index[1])
    energy = np.einsum("nf,hfg->hng", x, np.asarray(w_proj_W, np.float64))
    radial = np.einsum("nf,hfg->hng", x, np.asarray(radial_W, np.float64))
    tang = np.einsum("nf,hfg->hng", x, np.asarray(tangential_W, np.float64))
    e_d = energy[:, snd] - energy[:, rcv]
    r_d = (radial[:, snd] - radial[:, rcv]).transpose(1, 0, 2)
    t_d = (tang[:, snd] - tang[:, rcv]).transpose(1, 0, 2)
    sp = lambda v: np.logaddexp(0.0, v)
    scale = sp(np.asarray(radial_distance_log_scale, np.float64))
    r_log = (np.einsum("hef,hf->eh", e_d, np.asarray(radial_score, np.float64))
             - scale * np.asarray(edge_len, np.float64)[:, None])
    temp = sp(np.asarray(radial_temp_bias, np.float64)[None, :]
              + np.asarray(radial_temp_weight, np.float64)[None, :]
              * np.asarray(edge_len, np.float64)[:, None])
    r_log = r_log / (temp + 1e-4)
    t_log = np.einsum("hef,hf->eh", e_d, np.asarray(tangential_score, np.float64))

    def seg_softmax(logits):
        m = np.full((N, H), -np.inf)
        np.maximum.at(m, rcv, logits)
        a = np.exp(logits - m[rcv])
        d = np.zeros((N, H))
        np.add.at(d, rcv, a)
        return a / d[rcv]

    r_a, t_a = seg_softmax(r_log), seg_softmax(t_log)
    r_msg = np.zeros((N, H, F)); np.add.at(r_msg, rcv, r_a[:, :, None] * r_d)
    t_msg = np.zeros((N, H, F)); np.add.at(t_msg, rcv, t_a[:, :, None] * t_d)
    out = np.nan_to_num((r_msg + t_msg).mean(axis=1))
    return (x + out @ np.asarray(w_out_W, np.float64)).astype(np.float32)


_PROG_CACHE = {}


def kernel(**inputs) -> np.ndarray:
    if not np.allclose(np.asarray(inputs["radial_temp_weight"]), 0.0):
        return _numpy_fallback(**inputs)

    _import_concourse()
    from concourse.bass_utils import run_bass_kernel_spmd

    NB, in_maps = _host_prep(
        inputs["x"], inputs["edge_index"], inputs["edge_len"],
        inputs["w_proj_W"], inputs["radial_W"], inputs["tangential_W"],
        inputs["radial_score"], inputs["tangential_score"],
        inputs["radial_distance_log_scale"], inputs["radial_temp_bias"],
        inputs["radial_temp_weight"], inputs["w_out_W"],
    )
    if NB not in _PROG_CACHE:
        _PROG_CACHE[NB] = _build_program(NB)
    nc = _PROG_CACHE[NB]

    core_ids = list(range(NCORES))
    res = run_bass_kernel_spmd(nc, in_maps, core_ids)
    outs = [np.asarray(res.results[c]["out"])[:RPC] for c in core_ids]
    return np.concatenate(outs, axis=0).astype(np.float32)


if __name__ == "__main__":
    rng = np.random.default_rng(0)
    print("kernel module ok")
